# revision 13
# baseline (speedup 1.0000x reference)
"""BitLinear (BitNet b1.58) forward kernel for Trainium2, 8 NeuronCores.

Computes  y = einsum('bsi,oi->bso', x, w_ste) + bias  where
  scale  = max(mean(|W|), 1e-8)
  w_q    = clip(round(W/scale), -1, 1)   (ternary, realized as a threshold:
           w_q = (w > scale/2) - (w < -scale/2), exactly equivalent under
           round-half-to-even)
  w_ste  = w_q * scale  (forward value)

Sharding: data-parallel over rows. Each core owns 2048 rows of x
(= one batch element) and the full weight. On device each core:
  phase A: abs-sums its own 1/8 row-slice of W (8 MiB), locally reduces to
           a single f32 (gpsimd partition reduce), then an AllGather of the
           8 per-core scalars (1-descriptor DMAs each way) + a local 8-way
           sum assembles the global abs-sum. A dummy AllGather issued at
           kernel start absorbs the collective bring-up barrier into the
           abs-sum window. The head is bound by inter-core launch skew plus
           one small-message AllGather.
  phase B: per 512-wide out-feature chunk: stream W f32 in 2-k-tile slabs,
           ternary-quantize each slab to fp8 in 2 DVE passes (negated; fixed
           up by multiplying the output with -scale), and immediately run the
           slab's matmuls for the first 8 row-tiles (8 PSUM banks accumulate
           K across slabs) so the PE ramps ~2 us after the scale arrives;
           the remaining 8 row-tiles then run from the fully-quantized chunk
           while the next chunk's slabs quantize. Scale + bias are applied
           on the PSUM drain. x is emitted after the scale stream + the
           first W slabs so the DMA queues serve the critical path first.

x is staged pre-transposed [in_f, rows] in fp16 (matmul needs the
contraction dim on partitions for both operands; W is staged transposed
[in_f, out_f] in f32 so quantization happens on device at full precision).
"""

import numpy as np

import concourse.tile as tile
import concourse.mybir as mybir
from concourse import bacc, bass_isa
from concourse.bass import ts
from concourse.bass_utils import run_bass_kernel_spmd

N_CORES = 8
IN_F = 4096
OUT_F = 4096
ROWS_PER_CORE = 2048
P = 128                   # SBUF partitions
KT = IN_F // P            # 32 k-tiles along contraction
MT = ROWS_PER_CORE // P   # 16 row-tiles per core
OCH = 512                 # out-feature chunk = matmul free dim
NCH = OUT_F // OCH        # 8 chunks
QS = 2                    # k-tiles per quantize slab
NSLAB = KT // QS          # 16 slabs per chunk
MH = 8                    # row-tiles in the slab-streamed first half

F32 = mybir.dt.float32
F16 = mybir.dt.float16
F8 = mybir.dt.float8e4

LAST_RESULTS = None
_NC_CACHE = {}

RG = [list(range(N_CORES))]
USE_RDMA = True


def _build():
    nc = bacc.Bacc(
        "TRN2", target_bir_lowering=False, debug=False, num_devices=N_CORES
    )
    xt = nc.dram_tensor(
        "xt", [IN_F, ROWS_PER_CORE], F16, kind="ExternalInput"
    ).ap()
    wt = nc.dram_tensor("wt", [IN_F, OUT_F], F32, kind="ExternalInput").ap()
    # per-core 1/8 slice of the weight rows, only for the sharded scale
    # reduction (the global abs-sum is assembled with an AllGather)
    ws = nc.dram_tensor(
        "ws", [OUT_F // N_CORES, IN_F], F32, kind="ExternalInput"
    ).ap()
    bias = nc.dram_tensor("bias", [1, OUT_F], F32, kind="ExternalInput").ap()
    # y is written fp16 (exact f32 values are recovered host-side only up to
    # 2^-11 relative — well inside the error budget) to halve output DMA
    y = nc.dram_tensor(
        "y", [ROWS_PER_CORE, OUT_F], F16, kind="ExternalOutput"
    ).ap()

    with tile.TileContext(nc) as tc:
        with (
            tc.tile_pool(name="xp", bufs=1) as xp,
            tc.tile_pool(name="redp", bufs=1) as redp,
            tc.tile_pool(name="ccd", bufs=1, space="DRAM") as dramp,
            tc.tile_pool(name="psum", bufs=8, space="PSUM") as pp,
        ):
            # ---- phase A: scale = max(mean(|W|), 1e-8) ----
            # Each core abs-sums its own 1/8 row-slice of W; an AllGather of
            # the 8 per-core totals assembles the global sum.
            NS = 8
            CW = IN_F // 2  # 2048 columns per tile
            partials = redp.tile([P, NS], F32)
            ws_r = ws.rearrange("(kt p) c -> p kt c", p=P)
            with tc.tile_pool(name="sw", bufs=4) as swp:
                for i in range(NS):
                    stile = swp.tile([P, CW], F32)
                    nc.sync.dma_start(
                        out=stile,
                        in_=ws_r[:, i // 2, (i % 2) * CW : (i % 2 + 1) * CW],
                    )
                    nc.vector.tensor_reduce(
                        out=partials[:, i : i + 1],
                        in_=stile,
                        axis=mybir.AxisListType.X,
                        op=mybir.AluOpType.add,
                        apply_absolute_value=True,
                    )
            acc = redp.tile([P, 1], F32)
            nc.vector.tensor_reduce(
                out=acc,
                in_=partials,
                axis=mybir.AxisListType.X,
                op=mybir.AluOpType.add,
            )
            if USE_RDMA:
                # Cross-core exchange of the per-partition partials via raw
                # remote DMA (SWDGE) — no ncfw collective, so the runtime's
                # ~50 us collective bring-up barrier and the ~20 us mesh
                # mechanics disappear entirely. Call i broadcasts this core's
                # acc into column i of the XOR-peer's xch (dest tpb = own^i);
                # every receiver column order is a permutation of the 8
                # senders, which a sum doesn't care about. Desc-gen is
                # dependency-free (runs under the abs-sum); the trigger
                # carries the RAW dep on acc. Each sender bumps the
                # receiver's xch_sem by 2 (16 lanes / 8 dest slots), so 8
                # senders -> wait for 16, attached directly to the reduce.
                xch_sem = nc.alloc_semaphore("xch_sem")
                rel_sem = nc.alloc_semaphore("rel_sem")
                xch = redp.tile([P, N_CORES], F32)
                for i in range(N_CORES):
                    rd: list = [None] * N_CORES
                    rd[i] = (0, i)
                    nc.gpsimd.remote_dma_broadcast(
                        out_ap=xch[:, i : i + 1],
                        in_ap=acc,
                        remote_sem=xch_sem,
                        local_sem=rel_sem,
                        rdests=rd,
                    )
                nc.gpsimd.trigger_dma(count=None)
                colsum = redp.tile([P, 1], F32)
                # The arrival wait (xch_sem >= 16) is attached AFTER the
                # TileContext exits: the Tile scheduling sim models only
                # local semaphore updates, so an in-context wait on a
                # remotely-incremented semaphore deadlocks it. Mutating the
                # scheduled instruction's wait list is safe — it only delays
                # this reduce at runtime until the peers' data has landed.
                rdma_gate = nc.vector.tensor_reduce(
                    out=colsum,
                    in_=xch,
                    axis=mybir.AxisListType.X,
                    op=mybir.AluOpType.add,
                )
                accg = redp.tile([P, 1], F32)
                nc.gpsimd.partition_all_reduce(
                    accg, colsum, channels=P, reduce_op=bass_isa.ReduceOp.add
                )
            else:
                # local 128-partition reduce BEFORE the collective so only a
                # single f32 crosses cores (1-descriptor DMAs each way)
                allsum = redp.tile([P, 1], F32)
                nc.gpsimd.partition_all_reduce(
                    allsum, acc, channels=P, reduce_op=bass_isa.ReduceOp.add
                )
                cc_in = dramp.tile([1, 1], F32)
                cc_out = dramp.tile([1, N_CORES], F32)
                nc.sync.dma_start(cc_in[:], allsum[0:1, :])
                nc.gpsimd.collective_compute(
                    "AllGather",
                    mybir.AluOpType.bypass,
                    replica_groups=RG,
                    ins=[cc_in.opt()],
                    outs=[cc_out.opt()],
                )
                gat = redp.tile([1, N_CORES], F32)
                nc.sync.dma_start(gat[:], cc_out[:])
                tot = redp.tile([1, 1], F32)
                nc.vector.tensor_reduce(
                    out=tot,
                    in_=gat,
                    axis=mybir.AxisListType.X,
                    op=mybir.AluOpType.add,
                )
                accg = redp.tile([P, 1], F32)
                nc.gpsimd.partition_broadcast(accg, tot, channels=P)

            scale_bc = redp.tile([P, 1], F32)
            nc.vector.tensor_scalar(
                out=scale_bc,
                in0=accg,
                scalar1=1.0 / float(IN_F * OUT_F),
                scalar2=1e-8,
                op0=mybir.AluOpType.mult,
                op1=mybir.AluOpType.max,
            )
            tpos = redp.tile([P, 1], F32)
            tneg = redp.tile([P, 1], F32)
            sneg = redp.tile([P, 1], F32)
            nc.vector.tensor_scalar_mul(tpos, scale_bc, 0.5)
            nc.vector.tensor_scalar_mul(tneg, scale_bc, -0.5)
            # wq is built NEGATED (2 DVE passes instead of 3), compensated by
            # multiplying the output with -scale
            nc.vector.tensor_scalar_mul(sneg, scale_bc, -1.0)

            # ---- phase B: quantize + matmul per out-feature chunk ----
            with (
                tc.tile_pool(name="wf", bufs=6) as wfp,
                tc.tile_pool(name="wq", bufs=2) as wqp,
                tc.tile_pool(name="bt", bufs=2) as btp,
                tc.tile_pool(name="bl", bufs=2) as blp,
                tc.tile_pool(name="yp", bufs=4) as yp,
            ):
                xsb = xp.tile([P, KT, ROWS_PER_CORE], F16)
                xt_r = xt.rearrange("(kt p) r -> p kt r", p=P)
                for j in range(NCH):
                    jo = j * OCH
                    wq = wqp.tile([P, KT, OCH], F8)

                    # bias: 1-descriptor load + on-chip partition broadcast
                    bl = blp.tile([1, OCH], F32)
                    nc.sync.dma_start(out=bl, in_=bias[0:1, jo : jo + OCH])
                    bt = btp.tile([P, OCH], F32)
                    nc.gpsimd.partition_broadcast(bt, bl, channels=P)

                    # first half: stream k-slabs; quantize then immediately
                    # matmul them for row-tiles 0..MH-1 (8 PSUM banks open)
                    ps_half = [
                        pp.tile([P, OCH], F32, name="psb") for _ in range(MH)
                    ]
                    for s in range(NSLAB):
                        wf = wfp.tile([P, QS, OCH], F32)
                        for q in range(QS):
                            i = s * QS + q
                            nc.sync.dma_start(
                                out=wf[:, q, :],
                                in_=wt[i * P : (i + 1) * P, jo : jo + OCH],
                            )
                        wq_slab = wq[:, s * QS : (s + 1) * QS, :]
                        # wq_slab = (w < -T) - (w > T)  ==  -ternary(w)
                        nc.vector.tensor_scalar(
                            out=wq_slab,
                            in0=wf,
                            scalar1=tpos,
                            scalar2=None,
                            op0=mybir.AluOpType.is_gt,
                        )
                        nc.vector.scalar_tensor_tensor(
                            out=wq_slab,
                            in0=wf,
                            scalar=tneg,
                            in1=wq_slab,
                            op0=mybir.AluOpType.is_lt,
                            op1=mybir.AluOpType.subtract,
                        )
                        if j == 0:
                            # x k-tiles emitted just-in-time per slab, right
                            # behind that slab's W loads in DMA-queue order:
                            # the scale stream and the early W slabs own the
                            # queue heads, and each slab's x arrives with it.
                            for q in range(QS):
                                i = s * QS + q
                                nc.sync.dma_start(
                                    out=xsb[:, i, :], in_=xt_r[:, i, :]
                                )
                        for m in range(MH):
                            for q in range(QS):
                                i = s * QS + q
                                nc.tensor.matmul(
                                    ps_half[m],
                                    xsb[:, i, ts(m, P)],
                                    wq[:, i, :],
                                    start=(i == 0),
                                    stop=(i == KT - 1),
                                )
                    for m in range(MH):
                        ysb = yp.tile([P, OCH], F16)
                        # fused drain: ysb = psum * (-scale) + bias
                        nc.vector.scalar_tensor_tensor(
                            out=ysb,
                            in0=ps_half[m],
                            scalar=sneg,
                            in1=bt,
                            op0=mybir.AluOpType.mult,
                            op1=mybir.AluOpType.add,
                        )
                        nc.sync.dma_start(
                            out=y[ts(m, P), jo : jo + OCH], in_=ysb
                        )

                    # second half: chunk fully quantized; pure PE while the
                    # next chunk's slabs quantize on DVE
                    for m in range(MH, MT):
                        ps = pp.tile([P, OCH], F32, name="psb")
                        for i in range(KT):
                            nc.tensor.matmul(
                                ps,
                                xsb[:, i, ts(m, P)],
                                wq[:, i, :],
                                start=(i == 0),
                                stop=(i == KT - 1),
                            )
                        ysb = yp.tile([P, OCH], F16)
                        nc.vector.scalar_tensor_tensor(
                            out=ysb,
                            in0=ps,
                            scalar=sneg,
                            in1=bt,
                            op0=mybir.AluOpType.mult,
                            op1=mybir.AluOpType.add,
                        )
                        nc.sync.dma_start(
                            out=y[ts(m, P), jo : jo + OCH], in_=ysb
                        )

    if USE_RDMA:
        # check=False: the tile sem-assignment already used the native wait
        # slot; the extra wait is appended and lowered via an event-semaphore
        # fuse by the Bacc lowering.
        rdma_gate.wait_op(xch_sem, 16, "sem-ge", check=False)
    nc.compile()
    return nc


def _get_nc():
    if "nc" not in _NC_CACHE:
        _NC_CACHE["nc"] = _build()
    return _NC_CACHE["nc"]


def kernel(x, weight, bias):
    global LAST_RESULTS
    x = np.asarray(x)
    weight = np.asarray(weight, dtype=np.float32)
    bias = np.asarray(bias, dtype=np.float32)
    b, s, _ = x.shape
    rows = b * s
    assert rows == N_CORES * ROWS_PER_CORE

    xf = np.ascontiguousarray(x.reshape(rows, IN_F).astype(np.float32))
    wt = np.ascontiguousarray(weight.T)  # [in_f, out_f] f32
    b2 = np.ascontiguousarray(bias.reshape(1, OUT_F))

    osl = OUT_F // N_CORES
    in_maps = []
    for c in range(N_CORES):
        xs = xf[c * ROWS_PER_CORE : (c + 1) * ROWS_PER_CORE]
        xtc = np.ascontiguousarray(xs.astype(np.float16).T)
        m = {
            "xt": xtc,
            "wt": wt,
            "bias": b2,
            "ws": np.ascontiguousarray(weight[c * osl : (c + 1) * osl, :]),
        }
        in_maps.append(m)

    nc = _get_nc()
    try:
        res = run_bass_kernel_spmd(nc, in_maps, core_ids=list(range(N_CORES)))
    except Exception:
        # transient device wedge (NRT_EXEC_UNIT_UNRECOVERABLE) — one retry
        import time

        time.sleep(5.0)
        res = run_bass_kernel_spmd(nc, in_maps, core_ids=list(range(N_CORES)))
    LAST_RESULTS = res
    y = np.concatenate(
        [res.results[c]["y"].astype(np.float32) for c in range(N_CORES)],
        axis=0,
    )
    return np.ascontiguousarray(y.reshape(b, s, OUT_F).astype(np.float32))


# revision 14
# speedup vs baseline: 5.3685x; 5.3685x over previous
"""BitLinear (BitNet b1.58) forward kernel for Trainium2, 8 NeuronCores.

Computes  y = einsum('bsi,oi->bso', x, w_ste) + bias  where
  scale  = max(mean(|W|), 1e-8)
  w_q    = clip(round(W/scale), -1, 1)   (ternary, realized as a threshold:
           w_q = (w > scale/2) - (w < -scale/2), exactly equivalent under
           round-half-to-even)
  w_ste  = w_q * scale  (forward value)

Sharding: data-parallel over rows. Each core owns 2048 rows of x
(= one batch element) and the full weight. On device each core:
  phase A: abs-sums its own 1/8 row-slice of W (8 MiB), locally reduces to
           a single f32 (gpsimd partition reduce), then an AllGather of the
           8 per-core scalars (1-descriptor DMAs each way) + a local 8-way
           sum assembles the global abs-sum. A dummy AllGather issued at
           kernel start absorbs the collective bring-up barrier into the
           abs-sum window. The head is bound by inter-core launch skew plus
           one small-message AllGather.
  phase B: per 512-wide out-feature chunk: stream W f32 in 2-k-tile slabs,
           ternary-quantize each slab to fp8 in 2 DVE passes (negated; fixed
           up by multiplying the output with -scale), and immediately run the
           slab's matmuls for the first 8 row-tiles (8 PSUM banks accumulate
           K across slabs) so the PE ramps ~2 us after the scale arrives;
           the remaining 8 row-tiles then run from the fully-quantized chunk
           while the next chunk's slabs quantize. Scale + bias are applied
           on the PSUM drain. x is emitted after the scale stream + the
           first W slabs so the DMA queues serve the critical path first.

x is staged pre-transposed [in_f, rows] in fp16 (matmul needs the
contraction dim on partitions for both operands; W is staged transposed
[in_f, out_f] in f32 so quantization happens on device at full precision).
"""

import numpy as np

import concourse.tile as tile
import concourse.mybir as mybir
from concourse import bacc, bass_isa
from concourse.bass import ts
from concourse.bass_utils import run_bass_kernel_spmd

N_CORES = 8
IN_F = 4096
OUT_F = 4096
ROWS_PER_CORE = 2048
P = 128                   # SBUF partitions
KT = IN_F // P            # 32 k-tiles along contraction
MT = ROWS_PER_CORE // P   # 16 row-tiles per core
OCH = 512                 # out-feature chunk = matmul free dim
NCH = OUT_F // OCH        # 8 chunks
QS = 2                    # k-tiles per quantize slab
NSLAB = KT // QS          # 16 slabs per chunk
MH = 8                    # row-tiles in the slab-streamed first half

F32 = mybir.dt.float32
F16 = mybir.dt.float16
F8 = mybir.dt.float8e4

LAST_RESULTS = None
_NC_CACHE = {}

RG = [list(range(N_CORES))]
USE_RDMA = True


def _build():
    nc = bacc.Bacc(
        "TRN2", target_bir_lowering=False, debug=False, num_devices=N_CORES
    )
    xt = nc.dram_tensor(
        "xt", [IN_F, ROWS_PER_CORE], F16, kind="ExternalInput"
    ).ap()
    wt = nc.dram_tensor("wt", [IN_F, OUT_F], F32, kind="ExternalInput").ap()
    # per-core 1/8 slice of the weight rows, only for the sharded scale
    # reduction (the global abs-sum is assembled with an AllGather)
    ws = nc.dram_tensor(
        "ws", [OUT_F // N_CORES, IN_F], F32, kind="ExternalInput"
    ).ap()
    bias = nc.dram_tensor("bias", [1, OUT_F], F32, kind="ExternalInput").ap()
    # y is written fp16 (exact f32 values are recovered host-side only up to
    # 2^-11 relative — well inside the error budget) to halve output DMA
    y = nc.dram_tensor(
        "y", [ROWS_PER_CORE, OUT_F], F16, kind="ExternalOutput"
    ).ap()

    with tile.TileContext(nc) as tc:
        with (
            tc.tile_pool(name="xp", bufs=1) as xp,
            tc.tile_pool(name="redp", bufs=1) as redp,
            tc.tile_pool(name="ccd", bufs=1, space="DRAM") as dramp,
            tc.tile_pool(name="psum", bufs=8, space="PSUM") as pp,
        ):
            if USE_RDMA:
                # Unconsumed dummy AllGather: its only job is to make the
                # NEFF contain a collective, which switches the runtime to a
                # coordinated gang launch (nrt_build_global_comm). Without
                # it the 8 cores dispatch ~200us+ apart and the remote-DMA
                # scale exchange eats multi-ms launch skew. It runs on the
                # CC stream in the background and gates nothing.
                dum_in = dramp.tile([1, 1], F32, name="dum_in")
                dum_out = dramp.tile([1, N_CORES], F32, name="dum_out")
                nc.gpsimd.collective_compute(
                    "AllGather",
                    mybir.AluOpType.bypass,
                    replica_groups=RG,
                    ins=[dum_in.opt()],
                    outs=[dum_out.opt()],
                )

            # ---- phase A: scale = max(mean(|W|), 1e-8) ----
            # Each core abs-sums its own 1/8 row-slice of W; an AllGather of
            # the 8 per-core totals assembles the global sum.
            NS = 8
            CW = IN_F // 2  # 2048 columns per tile
            partials = redp.tile([P, NS], F32)
            ws_r = ws.rearrange("(kt p) c -> p kt c", p=P)
            with tc.tile_pool(name="sw", bufs=4) as swp:
                for i in range(NS):
                    stile = swp.tile([P, CW], F32)
                    nc.sync.dma_start(
                        out=stile,
                        in_=ws_r[:, i // 2, (i % 2) * CW : (i % 2 + 1) * CW],
                    )
                    nc.vector.tensor_reduce(
                        out=partials[:, i : i + 1],
                        in_=stile,
                        axis=mybir.AxisListType.X,
                        op=mybir.AluOpType.add,
                        apply_absolute_value=True,
                    )
            acc = redp.tile([P, 1], F32)
            nc.vector.tensor_reduce(
                out=acc,
                in_=partials,
                axis=mybir.AxisListType.X,
                op=mybir.AluOpType.add,
            )
            if USE_RDMA:
                # Cross-core exchange of the per-partition partials via raw
                # remote DMA (SWDGE) — no ncfw collective, so the runtime's
                # ~50 us collective bring-up barrier and the ~20 us mesh
                # mechanics disappear entirely. Call i broadcasts this core's
                # acc into column i of the XOR-peer's xch (dest tpb = own^i);
                # every receiver column order is a permutation of the 8
                # senders, which a sum doesn't care about. Desc-gen is
                # dependency-free (runs under the abs-sum); the trigger
                # carries the RAW dep on acc. Each sender bumps the
                # receiver's xch_sem by 2 (16 lanes / 8 dest slots), so 8
                # senders -> wait for 16, attached directly to the reduce.
                xch_sem = nc.alloc_semaphore("xch_sem")
                rel_sem = nc.alloc_semaphore("rel_sem")
                xch = redp.tile([P, N_CORES], F32)
                for i in range(N_CORES):
                    rd: list = [None] * N_CORES
                    rd[i] = (0, i)
                    nc.gpsimd.remote_dma_broadcast(
                        out_ap=xch[:, i : i + 1],
                        in_ap=acc,
                        remote_sem=xch_sem,
                        local_sem=rel_sem,
                        rdests=rd,
                    )
                nc.gpsimd.trigger_dma(count=None)
                colsum = redp.tile([P, 1], F32)
                # The arrival wait (xch_sem >= 16) is attached AFTER the
                # TileContext exits: the Tile scheduling sim models only
                # local semaphore updates, so an in-context wait on a
                # remotely-incremented semaphore deadlocks it. Mutating the
                # scheduled instruction's wait list is safe — it only delays
                # this reduce at runtime until the peers' data has landed.
                rdma_gate = nc.vector.tensor_reduce(
                    out=colsum,
                    in_=xch,
                    axis=mybir.AxisListType.X,
                    op=mybir.AluOpType.add,
                )
                accg = redp.tile([P, 1], F32)
                nc.gpsimd.partition_all_reduce(
                    accg, colsum, channels=P, reduce_op=bass_isa.ReduceOp.add
                )
            else:
                # local 128-partition reduce BEFORE the collective so only a
                # single f32 crosses cores (1-descriptor DMAs each way)
                allsum = redp.tile([P, 1], F32)
                nc.gpsimd.partition_all_reduce(
                    allsum, acc, channels=P, reduce_op=bass_isa.ReduceOp.add
                )
                cc_in = dramp.tile([1, 1], F32)
                cc_out = dramp.tile([1, N_CORES], F32)
                nc.sync.dma_start(cc_in[:], allsum[0:1, :])
                nc.gpsimd.collective_compute(
                    "AllGather",
                    mybir.AluOpType.bypass,
                    replica_groups=RG,
                    ins=[cc_in.opt()],
                    outs=[cc_out.opt()],
                )
                gat = redp.tile([1, N_CORES], F32)
                nc.sync.dma_start(gat[:], cc_out[:])
                tot = redp.tile([1, 1], F32)
                nc.vector.tensor_reduce(
                    out=tot,
                    in_=gat,
                    axis=mybir.AxisListType.X,
                    op=mybir.AluOpType.add,
                )
                accg = redp.tile([P, 1], F32)
                nc.gpsimd.partition_broadcast(accg, tot, channels=P)

            scale_bc = redp.tile([P, 1], F32)
            nc.vector.tensor_scalar(
                out=scale_bc,
                in0=accg,
                scalar1=1.0 / float(IN_F * OUT_F),
                scalar2=1e-8,
                op0=mybir.AluOpType.mult,
                op1=mybir.AluOpType.max,
            )
            tpos = redp.tile([P, 1], F32)
            tneg = redp.tile([P, 1], F32)
            sneg = redp.tile([P, 1], F32)
            nc.vector.tensor_scalar_mul(tpos, scale_bc, 0.5)
            nc.vector.tensor_scalar_mul(tneg, scale_bc, -0.5)
            # wq is built NEGATED (2 DVE passes instead of 3), compensated by
            # multiplying the output with -scale
            nc.vector.tensor_scalar_mul(sneg, scale_bc, -1.0)

            # ---- phase B: quantize + matmul per out-feature chunk ----
            with (
                tc.tile_pool(name="wf", bufs=6) as wfp,
                tc.tile_pool(name="wq", bufs=2) as wqp,
                tc.tile_pool(name="bt", bufs=2) as btp,
                tc.tile_pool(name="bl", bufs=2) as blp,
                tc.tile_pool(name="yp", bufs=4) as yp,
            ):
                xsb = xp.tile([P, KT, ROWS_PER_CORE], F16)
                xt_r = xt.rearrange("(kt p) r -> p kt r", p=P)
                for j in range(NCH):
                    jo = j * OCH
                    wq = wqp.tile([P, KT, OCH], F8)

                    # bias: 1-descriptor load + on-chip partition broadcast
                    bl = blp.tile([1, OCH], F32)
                    nc.sync.dma_start(out=bl, in_=bias[0:1, jo : jo + OCH])
                    bt = btp.tile([P, OCH], F32)
                    nc.gpsimd.partition_broadcast(bt, bl, channels=P)

                    # first half: stream k-slabs; quantize then immediately
                    # matmul them for row-tiles 0..MH-1 (8 PSUM banks open)
                    ps_half = [
                        pp.tile([P, OCH], F32, name="psb") for _ in range(MH)
                    ]
                    for s in range(NSLAB):
                        wf = wfp.tile([P, QS, OCH], F32)
                        for q in range(QS):
                            i = s * QS + q
                            nc.sync.dma_start(
                                out=wf[:, q, :],
                                in_=wt[i * P : (i + 1) * P, jo : jo + OCH],
                            )
                        wq_slab = wq[:, s * QS : (s + 1) * QS, :]
                        # wq_slab = (w < -T) - (w > T)  ==  -ternary(w)
                        nc.vector.tensor_scalar(
                            out=wq_slab,
                            in0=wf,
                            scalar1=tpos,
                            scalar2=None,
                            op0=mybir.AluOpType.is_gt,
                        )
                        nc.vector.scalar_tensor_tensor(
                            out=wq_slab,
                            in0=wf,
                            scalar=tneg,
                            in1=wq_slab,
                            op0=mybir.AluOpType.is_lt,
                            op1=mybir.AluOpType.subtract,
                        )
                        if j == 0:
                            # x k-tiles emitted just-in-time per slab, right
                            # behind that slab's W loads in DMA-queue order:
                            # the scale stream and the early W slabs own the
                            # queue heads, and each slab's x arrives with it.
                            for q in range(QS):
                                i = s * QS + q
                                nc.sync.dma_start(
                                    out=xsb[:, i, :], in_=xt_r[:, i, :]
                                )
                        for m in range(MH):
                            for q in range(QS):
                                i = s * QS + q
                                nc.tensor.matmul(
                                    ps_half[m],
                                    xsb[:, i, ts(m, P)],
                                    wq[:, i, :],
                                    start=(i == 0),
                                    stop=(i == KT - 1),
                                )
                    for m in range(MH):
                        ysb = yp.tile([P, OCH], F16)
                        # fused drain: ysb = psum * (-scale) + bias
                        nc.vector.scalar_tensor_tensor(
                            out=ysb,
                            in0=ps_half[m],
                            scalar=sneg,
                            in1=bt,
                            op0=mybir.AluOpType.mult,
                            op1=mybir.AluOpType.add,
                        )
                        nc.sync.dma_start(
                            out=y[ts(m, P), jo : jo + OCH], in_=ysb
                        )

                    # second half: chunk fully quantized; pure PE while the
                    # next chunk's slabs quantize on DVE
                    for m in range(MH, MT):
                        ps = pp.tile([P, OCH], F32, name="psb")
                        for i in range(KT):
                            nc.tensor.matmul(
                                ps,
                                xsb[:, i, ts(m, P)],
                                wq[:, i, :],
                                start=(i == 0),
                                stop=(i == KT - 1),
                            )
                        ysb = yp.tile([P, OCH], F16)
                        nc.vector.scalar_tensor_tensor(
                            out=ysb,
                            in0=ps,
                            scalar=sneg,
                            in1=bt,
                            op0=mybir.AluOpType.mult,
                            op1=mybir.AluOpType.add,
                        )
                        nc.sync.dma_start(
                            out=y[ts(m, P), jo : jo + OCH], in_=ysb
                        )

    if USE_RDMA:
        # check=False: the tile sem-assignment already used the native wait
        # slot; the extra wait is appended and lowered via an event-semaphore
        # fuse by the Bacc lowering.
        rdma_gate.wait_op(xch_sem, 16, "sem-ge", check=False)
    nc.compile()
    return nc


def _get_nc():
    if "nc" not in _NC_CACHE:
        _NC_CACHE["nc"] = _build()
    return _NC_CACHE["nc"]


def kernel(x, weight, bias):
    global LAST_RESULTS
    x = np.asarray(x)
    weight = np.asarray(weight, dtype=np.float32)
    bias = np.asarray(bias, dtype=np.float32)
    b, s, _ = x.shape
    rows = b * s
    assert rows == N_CORES * ROWS_PER_CORE

    xf = np.ascontiguousarray(x.reshape(rows, IN_F).astype(np.float32))
    wt = np.ascontiguousarray(weight.T)  # [in_f, out_f] f32
    b2 = np.ascontiguousarray(bias.reshape(1, OUT_F))

    osl = OUT_F // N_CORES
    in_maps = []
    for c in range(N_CORES):
        xs = xf[c * ROWS_PER_CORE : (c + 1) * ROWS_PER_CORE]
        xtc = np.ascontiguousarray(xs.astype(np.float16).T)
        m = {
            "xt": xtc,
            "wt": wt,
            "bias": b2,
            "ws": np.ascontiguousarray(weight[c * osl : (c + 1) * osl, :]),
        }
        in_maps.append(m)

    nc = _get_nc()
    try:
        res = run_bass_kernel_spmd(nc, in_maps, core_ids=list(range(N_CORES)))
    except Exception:
        # transient device wedge (NRT_EXEC_UNIT_UNRECOVERABLE) — one retry
        import time

        time.sleep(5.0)
        res = run_bass_kernel_spmd(nc, in_maps, core_ids=list(range(N_CORES)))
    LAST_RESULTS = res
    y = np.concatenate(
        [res.results[c]["y"].astype(np.float32) for c in range(N_CORES)],
        axis=0,
    )
    return np.ascontiguousarray(y.reshape(b, s, OUT_F).astype(np.float32))


# revision 15
# speedup vs baseline: 5.4754x; 1.0199x over previous
"""BitLinear (BitNet b1.58) forward kernel for Trainium2, 8 NeuronCores.

Computes  y = einsum('bsi,oi->bso', x, w_ste) + bias  where
  scale  = max(mean(|W|), 1e-8)
  w_q    = clip(round(W/scale), -1, 1)   (ternary, realized as a threshold:
           w_q = (w > scale/2) - (w < -scale/2), exactly equivalent under
           round-half-to-even)
  w_ste  = w_q * scale  (forward value)

Sharding: data-parallel over rows. Each core owns 2048 rows of x
(= one batch element) and the full weight. On device each core:
  phase A: abs-sums its own 1/8 row-slice of W (8 MiB), locally reduces to
           a single f32 (gpsimd partition reduce), then an AllGather of the
           8 per-core scalars (1-descriptor DMAs each way) + a local 8-way
           sum assembles the global abs-sum. A dummy AllGather issued at
           kernel start absorbs the collective bring-up barrier into the
           abs-sum window. The head is bound by inter-core launch skew plus
           one small-message AllGather.
  phase B: per 512-wide out-feature chunk: stream W f32 in 2-k-tile slabs,
           ternary-quantize each slab to fp8 in 2 DVE passes (negated; fixed
           up by multiplying the output with -scale), and immediately run the
           slab's matmuls for the first 8 row-tiles (8 PSUM banks accumulate
           K across slabs) so the PE ramps ~2 us after the scale arrives;
           the remaining 8 row-tiles then run from the fully-quantized chunk
           while the next chunk's slabs quantize. Scale + bias are applied
           on the PSUM drain. x is emitted after the scale stream + the
           first W slabs so the DMA queues serve the critical path first.

x is staged pre-transposed [in_f, rows] in fp16 (matmul needs the
contraction dim on partitions for both operands; W is staged transposed
[in_f, out_f] in f32 so quantization happens on device at full precision).
"""

import numpy as np

import concourse.tile as tile
import concourse.mybir as mybir
from concourse import bacc, bass_isa
from concourse.bass import ts
from concourse.bass_utils import run_bass_kernel_spmd

N_CORES = 8
IN_F = 4096
OUT_F = 4096
ROWS_PER_CORE = 2048
P = 128                   # SBUF partitions
KT = IN_F // P            # 32 k-tiles along contraction
MT = ROWS_PER_CORE // P   # 16 row-tiles per core
OCH = 512                 # out-feature chunk = matmul free dim
NCH = OUT_F // OCH        # 8 chunks
QS = 2                    # k-tiles per quantize slab
NSLAB = KT // QS          # 16 slabs per chunk
MH = 8                    # row-tiles in the slab-streamed first half

F32 = mybir.dt.float32
F16 = mybir.dt.float16
F8 = mybir.dt.float8e4

LAST_RESULTS = None
_NC_CACHE = {}

RG = [list(range(N_CORES))]
# Raw SWDGE remote-DMA scale exchange (kept for reference): it works and is
# correct, but its completion is gated on every core reaching trigger_dma at
# launch+~60us with run-variable gang skew, which measured no better than the
# small AllGather (whose gating trigger fires at launch+~21us). It also NEEDS
# a (dummy) collective in the NEFF for the coordinated gang launch — without
# one, core dispatch staggers by 0.2-5 ms and the exchange eats all of it.
USE_RDMA = False


def _build():
    nc = bacc.Bacc(
        "TRN2", target_bir_lowering=False, debug=False, num_devices=N_CORES
    )
    xt = nc.dram_tensor(
        "xt", [IN_F, ROWS_PER_CORE], F16, kind="ExternalInput"
    ).ap()
    wt = nc.dram_tensor("wt", [IN_F, OUT_F], F32, kind="ExternalInput").ap()
    # per-core 1/8 slice of the weight rows, only for the sharded scale
    # reduction (the global abs-sum is assembled with an AllGather)
    ws = nc.dram_tensor(
        "ws", [OUT_F // N_CORES, IN_F], F32, kind="ExternalInput"
    ).ap()
    bias = nc.dram_tensor("bias", [1, OUT_F], F32, kind="ExternalInput").ap()
    # y is written fp16 (exact f32 values are recovered host-side only up to
    # 2^-11 relative — well inside the error budget) to halve output DMA
    y = nc.dram_tensor(
        "y", [ROWS_PER_CORE, OUT_F], F16, kind="ExternalOutput"
    ).ap()

    with tile.TileContext(nc) as tc:
        with (
            tc.tile_pool(name="xp", bufs=1) as xp,
            tc.tile_pool(name="redp", bufs=1) as redp,
            tc.tile_pool(name="ccd", bufs=1, space="DRAM") as dramp,
            tc.tile_pool(name="psum", bufs=8, space="PSUM") as pp,
        ):
            if USE_RDMA:
                # Unconsumed dummy AllGather: its only job is to make the
                # NEFF contain a collective, which switches the runtime to a
                # coordinated gang launch (nrt_build_global_comm). Without
                # it the 8 cores dispatch ~200us+ apart and the remote-DMA
                # scale exchange eats multi-ms launch skew. It runs on the
                # CC stream in the background and gates nothing.
                dum_in = dramp.tile([1, 1], F32, name="dum_in")
                dum_out = dramp.tile([1, N_CORES], F32, name="dum_out")
                nc.gpsimd.collective_compute(
                    "AllGather",
                    mybir.AluOpType.bypass,
                    replica_groups=RG,
                    ins=[dum_in.opt()],
                    outs=[dum_out.opt()],
                )

            # ---- phase A: scale = max(mean(|W|), 1e-8) ----
            # Each core abs-sums its own 1/8 row-slice of W; an AllGather of
            # the 8 per-core totals assembles the global sum.
            NS = 8
            CW = IN_F // 2  # 2048 columns per tile
            partials = redp.tile([P, NS], F32)
            ws_r = ws.rearrange("(kt p) c -> p kt c", p=P)
            with tc.tile_pool(name="sw", bufs=4) as swp:
                for i in range(NS):
                    stile = swp.tile([P, CW], F32)
                    nc.sync.dma_start(
                        out=stile,
                        in_=ws_r[:, i // 2, (i % 2) * CW : (i % 2 + 1) * CW],
                    )
                    nc.vector.tensor_reduce(
                        out=partials[:, i : i + 1],
                        in_=stile,
                        axis=mybir.AxisListType.X,
                        op=mybir.AluOpType.add,
                        apply_absolute_value=True,
                    )
            acc = redp.tile([P, 1], F32)
            nc.vector.tensor_reduce(
                out=acc,
                in_=partials,
                axis=mybir.AxisListType.X,
                op=mybir.AluOpType.add,
            )
            if USE_RDMA:
                # Cross-core exchange of the per-partition partials via raw
                # remote DMA (SWDGE) — no ncfw collective, so the runtime's
                # ~50 us collective bring-up barrier and the ~20 us mesh
                # mechanics disappear entirely. Call i broadcasts this core's
                # acc into column i of the XOR-peer's xch (dest tpb = own^i);
                # every receiver column order is a permutation of the 8
                # senders, which a sum doesn't care about. Desc-gen is
                # dependency-free (runs under the abs-sum); the trigger
                # carries the RAW dep on acc. Each sender bumps the
                # receiver's xch_sem by 2 (16 lanes / 8 dest slots), so 8
                # senders -> wait for 16, attached directly to the reduce.
                xch_sem = nc.alloc_semaphore("xch_sem")
                rel_sem = nc.alloc_semaphore("rel_sem")
                xch = redp.tile([P, N_CORES], F32)
                for i in range(N_CORES):
                    rd: list = [None] * N_CORES
                    rd[i] = (0, i)
                    nc.gpsimd.remote_dma_broadcast(
                        out_ap=xch[:, i : i + 1],
                        in_ap=acc,
                        remote_sem=xch_sem,
                        local_sem=rel_sem,
                        rdests=rd,
                    )
                nc.gpsimd.trigger_dma(count=None)
                colsum = redp.tile([P, 1], F32)
                # The arrival wait (xch_sem >= 16) is attached AFTER the
                # TileContext exits: the Tile scheduling sim models only
                # local semaphore updates, so an in-context wait on a
                # remotely-incremented semaphore deadlocks it. Mutating the
                # scheduled instruction's wait list is safe — it only delays
                # this reduce at runtime until the peers' data has landed.
                rdma_gate = nc.vector.tensor_reduce(
                    out=colsum,
                    in_=xch,
                    axis=mybir.AxisListType.X,
                    op=mybir.AluOpType.add,
                )
                accg = redp.tile([P, 1], F32)
                nc.gpsimd.partition_all_reduce(
                    accg, colsum, channels=P, reduce_op=bass_isa.ReduceOp.add
                )
            else:
                # local 128-partition reduce BEFORE the collective so only a
                # single f32 crosses cores (1-descriptor DMAs each way)
                allsum = redp.tile([P, 1], F32)
                nc.gpsimd.partition_all_reduce(
                    allsum, acc, channels=P, reduce_op=bass_isa.ReduceOp.add
                )
                cc_in = dramp.tile([1, 1], F32)
                cc_out = dramp.tile([1, N_CORES], F32)
                nc.sync.dma_start(cc_in[:], allsum[0:1, :])
                nc.gpsimd.collective_compute(
                    "AllGather",
                    mybir.AluOpType.bypass,
                    replica_groups=RG,
                    ins=[cc_in.opt()],
                    outs=[cc_out.opt()],
                )
                gat = redp.tile([1, N_CORES], F32)
                nc.sync.dma_start(gat[:], cc_out[:])
                tot = redp.tile([1, 1], F32)
                nc.vector.tensor_reduce(
                    out=tot,
                    in_=gat,
                    axis=mybir.AxisListType.X,
                    op=mybir.AluOpType.add,
                )
                accg = redp.tile([P, 1], F32)
                nc.gpsimd.partition_broadcast(accg, tot, channels=P)

            scale_bc = redp.tile([P, 1], F32)
            nc.vector.tensor_scalar(
                out=scale_bc,
                in0=accg,
                scalar1=1.0 / float(IN_F * OUT_F),
                scalar2=1e-8,
                op0=mybir.AluOpType.mult,
                op1=mybir.AluOpType.max,
            )
            tpos = redp.tile([P, 1], F32)
            tneg = redp.tile([P, 1], F32)
            sneg = redp.tile([P, 1], F32)
            nc.vector.tensor_scalar_mul(tpos, scale_bc, 0.5)
            nc.vector.tensor_scalar_mul(tneg, scale_bc, -0.5)
            # wq is built NEGATED (2 DVE passes instead of 3), compensated by
            # multiplying the output with -scale
            nc.vector.tensor_scalar_mul(sneg, scale_bc, -1.0)

            # ---- phase B: quantize + matmul per out-feature chunk ----
            with (
                tc.tile_pool(name="wf", bufs=6) as wfp,
                tc.tile_pool(name="wq", bufs=2) as wqp,
                tc.tile_pool(name="bt", bufs=2) as btp,
                tc.tile_pool(name="bl", bufs=2) as blp,
                tc.tile_pool(name="yp", bufs=4) as yp,
            ):
                xsb = xp.tile([P, KT, ROWS_PER_CORE], F16)
                xt_r = xt.rearrange("(kt p) r -> p kt r", p=P)
                for j in range(NCH):
                    jo = j * OCH
                    wq = wqp.tile([P, KT, OCH], F8)

                    # bias: 1-descriptor load + on-chip partition broadcast
                    bl = blp.tile([1, OCH], F32)
                    nc.sync.dma_start(out=bl, in_=bias[0:1, jo : jo + OCH])
                    bt = btp.tile([P, OCH], F32)
                    nc.gpsimd.partition_broadcast(bt, bl, channels=P)

                    # first half: stream k-slabs; quantize then immediately
                    # matmul them for row-tiles 0..MH-1 (8 PSUM banks open)
                    ps_half = [
                        pp.tile([P, OCH], F32, name="psb") for _ in range(MH)
                    ]
                    for s in range(NSLAB):
                        wf = wfp.tile([P, QS, OCH], F32)
                        for q in range(QS):
                            i = s * QS + q
                            nc.sync.dma_start(
                                out=wf[:, q, :],
                                in_=wt[i * P : (i + 1) * P, jo : jo + OCH],
                            )
                        wq_slab = wq[:, s * QS : (s + 1) * QS, :]
                        # wq_slab = (w < -T) - (w > T)  ==  -ternary(w)
                        nc.vector.tensor_scalar(
                            out=wq_slab,
                            in0=wf,
                            scalar1=tpos,
                            scalar2=None,
                            op0=mybir.AluOpType.is_gt,
                        )
                        nc.vector.scalar_tensor_tensor(
                            out=wq_slab,
                            in0=wf,
                            scalar=tneg,
                            in1=wq_slab,
                            op0=mybir.AluOpType.is_lt,
                            op1=mybir.AluOpType.subtract,
                        )
                        if j == 0:
                            # x k-tiles emitted just-in-time per slab, right
                            # behind that slab's W loads in DMA-queue order:
                            # the scale stream and the early W slabs own the
                            # queue heads, and each slab's x arrives with it.
                            for q in range(QS):
                                i = s * QS + q
                                nc.sync.dma_start(
                                    out=xsb[:, i, :], in_=xt_r[:, i, :]
                                )
                        for m in range(MH):
                            for q in range(QS):
                                i = s * QS + q
                                nc.tensor.matmul(
                                    ps_half[m],
                                    xsb[:, i, ts(m, P)],
                                    wq[:, i, :],
                                    start=(i == 0),
                                    stop=(i == KT - 1),
                                )
                    for m in range(MH):
                        ysb = yp.tile([P, OCH], F16)
                        # fused drain: ysb = psum * (-scale) + bias
                        nc.vector.scalar_tensor_tensor(
                            out=ysb,
                            in0=ps_half[m],
                            scalar=sneg,
                            in1=bt,
                            op0=mybir.AluOpType.mult,
                            op1=mybir.AluOpType.add,
                        )
                        nc.sync.dma_start(
                            out=y[ts(m, P), jo : jo + OCH], in_=ysb
                        )

                    # second half: chunk fully quantized; pure PE while the
                    # next chunk's slabs quantize on DVE
                    for m in range(MH, MT):
                        ps = pp.tile([P, OCH], F32, name="psb")
                        for i in range(KT):
                            nc.tensor.matmul(
                                ps,
                                xsb[:, i, ts(m, P)],
                                wq[:, i, :],
                                start=(i == 0),
                                stop=(i == KT - 1),
                            )
                        ysb = yp.tile([P, OCH], F16)
                        nc.vector.scalar_tensor_tensor(
                            out=ysb,
                            in0=ps,
                            scalar=sneg,
                            in1=bt,
                            op0=mybir.AluOpType.mult,
                            op1=mybir.AluOpType.add,
                        )
                        nc.sync.dma_start(
                            out=y[ts(m, P), jo : jo + OCH], in_=ysb
                        )

    if USE_RDMA:
        # check=False: the tile sem-assignment already used the native wait
        # slot; the extra wait is appended and lowered via an event-semaphore
        # fuse by the Bacc lowering.
        rdma_gate.wait_op(xch_sem, 16, "sem-ge", check=False)
    nc.compile()
    return nc


def _get_nc():
    if "nc" not in _NC_CACHE:
        _NC_CACHE["nc"] = _build()
    return _NC_CACHE["nc"]


def kernel(x, weight, bias):
    global LAST_RESULTS
    x = np.asarray(x)
    weight = np.asarray(weight, dtype=np.float32)
    bias = np.asarray(bias, dtype=np.float32)
    b, s, _ = x.shape
    rows = b * s
    assert rows == N_CORES * ROWS_PER_CORE

    xf = np.ascontiguousarray(x.reshape(rows, IN_F).astype(np.float32))
    wt = np.ascontiguousarray(weight.T)  # [in_f, out_f] f32
    b2 = np.ascontiguousarray(bias.reshape(1, OUT_F))

    osl = OUT_F // N_CORES
    in_maps = []
    for c in range(N_CORES):
        xs = xf[c * ROWS_PER_CORE : (c + 1) * ROWS_PER_CORE]
        xtc = np.ascontiguousarray(xs.astype(np.float16).T)
        m = {
            "xt": xtc,
            "wt": wt,
            "bias": b2,
            "ws": np.ascontiguousarray(weight[c * osl : (c + 1) * osl, :]),
        }
        in_maps.append(m)

    nc = _get_nc()
    try:
        res = run_bass_kernel_spmd(nc, in_maps, core_ids=list(range(N_CORES)))
    except Exception:
        # transient device wedge (NRT_EXEC_UNIT_UNRECOVERABLE) — one retry
        import time

        time.sleep(5.0)
        res = run_bass_kernel_spmd(nc, in_maps, core_ids=list(range(N_CORES)))
    LAST_RESULTS = res
    y = np.concatenate(
        [res.results[c]["y"].astype(np.float32) for c in range(N_CORES)],
        axis=0,
    )
    return np.ascontiguousarray(y.reshape(b, s, OUT_F).astype(np.float32))


# revision 24
# speedup vs baseline: 5.5440x; 1.0125x over previous
"""BitLinear (BitNet b1.58) forward kernel for Trainium2, 8 NeuronCores.

Computes  y = einsum('bsi,oi->bso', x, w_ste) + bias  where
  scale  = max(mean(|W|), 1e-8)
  w_q    = clip(round(W/scale), -1, 1)   (ternary, realized as a threshold:
           w_q = (w > scale/2) - (w < -scale/2), exactly equivalent under
           round-half-to-even)
  w_ste  = w_q * scale  (forward value)

Sharding: data-parallel over rows. Each core owns 2048 rows of x
(= one batch element) and the full weight. On device each core:
  phase A: abs-sums its own 1/8 row-slice of W (8 MiB), locally reduces to
           a single f32 (gpsimd partition reduce), then an AllGather of the
           8 per-core scalars (1-descriptor DMAs each way) + a local 8-way
           sum assembles the global abs-sum. A dummy AllGather issued at
           kernel start absorbs the collective bring-up barrier into the
           abs-sum window. The head is bound by inter-core launch skew plus
           one small-message AllGather.
  phase B: per 512-wide out-feature chunk: stream W f32 in 2-k-tile slabs,
           ternary-quantize each slab to fp8 in 2 DVE passes (negated; fixed
           up by multiplying the output with -scale), and immediately run the
           slab's matmuls for the first 8 row-tiles (8 PSUM banks accumulate
           K across slabs) so the PE ramps ~2 us after the scale arrives;
           the remaining 8 row-tiles then run from the fully-quantized chunk
           while the next chunk's slabs quantize. Scale + bias are applied
           on the PSUM drain. x is emitted after the scale stream + the
           first W slabs so the DMA queues serve the critical path first.

x is staged pre-transposed [in_f, rows] in fp16 (matmul needs the
contraction dim on partitions for both operands; W is staged transposed
[in_f, out_f] in f32 so quantization happens on device at full precision).
"""

import numpy as np

import concourse.tile as tile
import concourse.mybir as mybir
from concourse import bacc, bass_isa
from concourse.bass import ts
from concourse.bass_utils import run_bass_kernel_spmd

N_CORES = 8
IN_F = 4096
OUT_F = 4096
ROWS_PER_CORE = 2048
P = 128                   # SBUF partitions
KT = IN_F // P            # 32 k-tiles along contraction
MT = ROWS_PER_CORE // P   # 16 row-tiles per core
OCH = 512                 # out-feature chunk = matmul free dim
NCH = OUT_F // OCH        # 8 chunks
QS = 2                    # k-tiles per quantize slab
NSLAB = KT // QS          # 16 slabs per chunk
MH = 8                    # row-tiles in the slab-streamed first half

F32 = mybir.dt.float32
F16 = mybir.dt.float16
F8 = mybir.dt.float8e4

LAST_RESULTS = None
_NC_CACHE = {}

RG = [list(range(N_CORES))]
# Raw SWDGE remote-DMA scale exchange (kept for reference): it works and is
# correct, but its completion is gated on every core reaching trigger_dma at
# launch+~60us with run-variable gang skew, which measured no better than the
# small AllGather (whose gating trigger fires at launch+~21us). It also NEEDS
# a (dummy) collective in the NEFF for the coordinated gang launch — without
# one, core dispatch staggers by 0.2-5 ms and the exchange eats all of it.
USE_RDMA = False


def _build():
    nc = bacc.Bacc(
        "TRN2", target_bir_lowering=False, debug=False, num_devices=N_CORES
    )
    xt = nc.dram_tensor(
        "xt", [IN_F, ROWS_PER_CORE], F16, kind="ExternalInput"
    ).ap()
    wt = nc.dram_tensor("wt", [IN_F, OUT_F], F32, kind="ExternalInput").ap()
    # per-core 1/8 slice of the weight rows, only for the sharded scale
    # reduction (the global abs-sum is assembled with an AllGather)
    ws = nc.dram_tensor(
        "ws", [OUT_F // N_CORES, IN_F], F32, kind="ExternalInput"
    ).ap()
    bias = nc.dram_tensor("bias", [1, OUT_F], F32, kind="ExternalInput").ap()
    # y is written fp16 (exact f32 values are recovered host-side only up to
    # 2^-11 relative — well inside the error budget) to halve output DMA
    y = nc.dram_tensor(
        "y", [ROWS_PER_CORE, OUT_F], F16, kind="ExternalOutput"
    ).ap()

    with tile.TileContext(nc) as tc:
        with (
            tc.tile_pool(name="xp", bufs=1) as xp,
            tc.tile_pool(name="redp", bufs=1) as redp,
            tc.tile_pool(name="ccd", bufs=1, space="DRAM") as dramp,
            tc.tile_pool(name="psum", bufs=8, space="PSUM") as pp,
        ):
            if USE_RDMA:
                # Desc-gen preps FIRST on the gpsimd queue: they are
                # dependency-free (the RAW dep on acc rides on trigger_dma),
                # so emitting them at the top runs them at ~6-14us instead of
                # ~50us — the exchange trigger then fires the moment the
                # abs-sum lands. Call i broadcasts this core's acc into
                # column i of the XOR-peer's xch (dest tpb = own^i); every
                # receiver's column order is a permutation of the 8 senders,
                # which the sum doesn't care about. Each sender bumps the
                # receiver's xch_sem by 2 (16 lanes / 8 dest slots), so 8
                # senders -> wait for 16.
                xch_sem = nc.alloc_semaphore("xch_sem")
                rel_sem = nc.alloc_semaphore("rel_sem")
                acc = redp.tile([P, 1], F32, name="acc")
                xch = redp.tile([P, N_CORES], F32, name="xch")
                for i in range(N_CORES):
                    rd: list = [None] * N_CORES
                    rd[i] = (0, i)
                    nc.gpsimd.remote_dma_broadcast(
                        out_ap=xch[:, i : i + 1],
                        in_ap=acc,
                        remote_sem=xch_sem,
                        local_sem=rel_sem,
                        rdests=rd,
                    )

            # ---- phase A: scale = max(mean(|W|), 1e-8) ----
            # Each core abs-sums its own 1/8 row-slice of W; an AllGather of
            # the 8 per-core totals assembles the global sum.
            NS = 8
            CW = IN_F // 2  # 2048 columns per tile
            partials = redp.tile([P, NS], F32)
            ws_r = ws.rearrange("(kt p) c -> p kt c", p=P)
            with tc.tile_pool(name="sw", bufs=4) as swp:
                for i in range(NS):
                    stile = swp.tile([P, CW], F32)
                    nc.sync.dma_start(
                        out=stile,
                        in_=ws_r[:, i // 2, (i % 2) * CW : (i % 2 + 1) * CW],
                    )
                    nc.vector.tensor_reduce(
                        out=partials[:, i : i + 1],
                        in_=stile,
                        axis=mybir.AxisListType.X,
                        op=mybir.AluOpType.add,
                        apply_absolute_value=True,
                    )
            if not USE_RDMA:
                acc = redp.tile([P, 1], F32, name="acc")
            nc.vector.tensor_reduce(
                out=acc,
                in_=partials,
                axis=mybir.AxisListType.X,
                op=mybir.AluOpType.add,
            )
            if USE_RDMA:
                # Fire the pre-generated exchange descriptors the instant the
                # abs-sum lands. The preps were emitted before acc had a
                # producer, so the usual deferred-RAW-on-trigger edge does
                # NOT exist; declaring acc in signals_writable gives the
                # trigger a real tile edge on the reduce (WAW), with no
                # semaphore games (a then_inc on the reduce or on a helper
                # DVE op overflows walrus's per-instruction sync-update
                # slots).
                nc.gpsimd.trigger_dma(count=None, signals_writable=[acc])
                # Unconsumed dummy AllGather, emitted AFTER the trigger so
                # its ncfw-readiness stall cannot block the gpsimd queue
                # ahead of the trigger. Its only job is to make the NEFF
                # contain a collective, which switches the runtime to a
                # coordinated gang launch (nrt_build_global_comm) — without
                # it, core dispatch staggers by 0.2-5ms and the exchange
                # eats all of it. It runs on the CC stream in the background
                # and gates nothing.
                dum_in = dramp.tile([1, 1], F32, name="dum_in")
                dum_out = dramp.tile([1, N_CORES], F32, name="dum_out")
                nc.gpsimd.collective_compute(
                    "AllGather",
                    mybir.AluOpType.bypass,
                    replica_groups=RG,
                    ins=[dum_in.opt()],
                    outs=[dum_out.opt()],
                )
                colsum = redp.tile([P, 1], F32)
                # The arrival wait (xch_sem >= 16) is attached AFTER the
                # TileContext exits: the Tile scheduling sim models only
                # local semaphore updates, so an in-context wait on a
                # remotely-incremented semaphore deadlocks it. Mutating the
                # scheduled instruction's wait list is safe — it only delays
                # this reduce at runtime until the peers' data has landed.
                rdma_gate = nc.vector.tensor_reduce(
                    out=colsum,
                    in_=xch,
                    axis=mybir.AxisListType.X,
                    op=mybir.AluOpType.add,
                )
                accg = redp.tile([P, 1], F32)
                nc.gpsimd.partition_all_reduce(
                    accg, colsum, channels=P, reduce_op=bass_isa.ReduceOp.add
                )
            else:
                # local 128-partition reduce BEFORE the collective so only a
                # single f32 crosses cores (1-descriptor DMAs each way)
                allsum = redp.tile([P, 1], F32)
                nc.gpsimd.partition_all_reduce(
                    allsum, acc, channels=P, reduce_op=bass_isa.ReduceOp.add
                )
                cc_in = dramp.tile([1, 1], F32)
                cc_out = dramp.tile([1, N_CORES], F32)
                nc.sync.dma_start(cc_in[:], allsum[0:1, :])
                nc.gpsimd.collective_compute(
                    "AllGather",
                    mybir.AluOpType.bypass,
                    replica_groups=RG,
                    ins=[cc_in.opt()],
                    outs=[cc_out.opt()],
                )
                gat = redp.tile([1, N_CORES], F32)
                nc.sync.dma_start(gat[:], cc_out[:])
                tot = redp.tile([1, 1], F32)
                nc.vector.tensor_reduce(
                    out=tot,
                    in_=gat,
                    axis=mybir.AxisListType.X,
                    op=mybir.AluOpType.add,
                )
                accg = redp.tile([P, 1], F32)
                nc.gpsimd.partition_broadcast(accg, tot, channels=P)

            scale_bc = redp.tile([P, 1], F32)
            nc.vector.tensor_scalar(
                out=scale_bc,
                in0=accg,
                scalar1=1.0 / float(IN_F * OUT_F),
                scalar2=1e-8,
                op0=mybir.AluOpType.mult,
                op1=mybir.AluOpType.max,
            )
            tpos = redp.tile([P, 1], F32)
            tneg = redp.tile([P, 1], F32)
            sneg = redp.tile([P, 1], F32)
            nc.vector.tensor_scalar_mul(tpos, scale_bc, 0.5)
            nc.vector.tensor_scalar_mul(tneg, scale_bc, -0.5)
            # wq is built NEGATED (2 DVE passes instead of 3), compensated by
            # multiplying the output with -scale
            nc.vector.tensor_scalar_mul(sneg, scale_bc, -1.0)

            # ---- phase B: quantize + matmul per out-feature chunk ----
            with (
                tc.tile_pool(name="wf", bufs=6) as wfp,
                tc.tile_pool(name="wq", bufs=2) as wqp,
                tc.tile_pool(name="bt", bufs=2) as btp,
                tc.tile_pool(name="bl", bufs=2) as blp,
                tc.tile_pool(name="yp", bufs=4) as yp,
            ):
                xsb = xp.tile([P, KT, ROWS_PER_CORE], F16)
                xt_r = xt.rearrange("(kt p) r -> p kt r", p=P)
                for j in range(NCH):
                    jo = j * OCH
                    wq = wqp.tile([P, KT, OCH], F8)

                    # bias: 1-descriptor load + on-chip partition broadcast
                    bl = blp.tile([1, OCH], F32)
                    nc.sync.dma_start(out=bl, in_=bias[0:1, jo : jo + OCH])
                    bt = btp.tile([P, OCH], F32)
                    nc.gpsimd.partition_broadcast(bt, bl, channels=P)

                    # first half: stream k-slabs; quantize then immediately
                    # matmul them for row-tiles 0..MH-1 (8 PSUM banks open)
                    ps_half = [
                        pp.tile([P, OCH], F32, name="psb") for _ in range(MH)
                    ]
                    for s in range(NSLAB):
                        wf = wfp.tile([P, QS, OCH], F32)
                        for q in range(QS):
                            i = s * QS + q
                            nc.sync.dma_start(
                                out=wf[:, q, :],
                                in_=wt[i * P : (i + 1) * P, jo : jo + OCH],
                            )
                        wq_slab = wq[:, s * QS : (s + 1) * QS, :]
                        # wq_slab = (w < -T) - (w > T)  ==  -ternary(w)
                        nc.vector.tensor_scalar(
                            out=wq_slab,
                            in0=wf,
                            scalar1=tpos,
                            scalar2=None,
                            op0=mybir.AluOpType.is_gt,
                        )
                        nc.vector.scalar_tensor_tensor(
                            out=wq_slab,
                            in0=wf,
                            scalar=tneg,
                            in1=wq_slab,
                            op0=mybir.AluOpType.is_lt,
                            op1=mybir.AluOpType.subtract,
                        )
                        if j == 0:
                            # x k-tiles emitted just-in-time per slab, right
                            # behind that slab's W loads in DMA-queue order:
                            # the scale stream and the early W slabs own the
                            # queue heads, and each slab's x arrives with it.
                            for q in range(QS):
                                i = s * QS + q
                                nc.sync.dma_start(
                                    out=xsb[:, i, :], in_=xt_r[:, i, :]
                                )
                        for m in range(MH):
                            for q in range(QS):
                                i = s * QS + q
                                nc.tensor.matmul(
                                    ps_half[m],
                                    xsb[:, i, ts(m, P)],
                                    wq[:, i, :],
                                    start=(i == 0),
                                    stop=(i == KT - 1),
                                )
                    for m in range(MH):
                        ysb = yp.tile([P, OCH], F16)
                        # fused drain: ysb = psum * (-scale) + bias
                        nc.vector.scalar_tensor_tensor(
                            out=ysb,
                            in0=ps_half[m],
                            scalar=sneg,
                            in1=bt,
                            op0=mybir.AluOpType.mult,
                            op1=mybir.AluOpType.add,
                        )
                        nc.sync.dma_start(
                            out=y[ts(m, P), jo : jo + OCH], in_=ysb
                        )

                    # second half: chunk fully quantized; pure PE while the
                    # next chunk's slabs quantize on DVE
                    for m in range(MH, MT):
                        ps = pp.tile([P, OCH], F32, name="psb")
                        for i in range(KT):
                            nc.tensor.matmul(
                                ps,
                                xsb[:, i, ts(m, P)],
                                wq[:, i, :],
                                start=(i == 0),
                                stop=(i == KT - 1),
                            )
                        ysb = yp.tile([P, OCH], F16)
                        nc.vector.scalar_tensor_tensor(
                            out=ysb,
                            in0=ps,
                            scalar=sneg,
                            in1=bt,
                            op0=mybir.AluOpType.mult,
                            op1=mybir.AluOpType.add,
                        )
                        nc.sync.dma_start(
                            out=y[ts(m, P), jo : jo + OCH], in_=ysb
                        )

    if USE_RDMA:
        # check=False: the tile sem-assignment already used the native wait
        # slot; the extra wait is appended and lowered via an event-semaphore
        # fuse by the Bacc lowering.
        rdma_gate.wait_op(xch_sem, 16, "sem-ge", check=False)
    nc.compile()
    return nc


def _get_nc():
    if "nc" not in _NC_CACHE:
        _NC_CACHE["nc"] = _build()
    return _NC_CACHE["nc"]


def kernel(x, weight, bias):
    global LAST_RESULTS
    x = np.asarray(x)
    weight = np.asarray(weight, dtype=np.float32)
    bias = np.asarray(bias, dtype=np.float32)
    b, s, _ = x.shape
    rows = b * s
    assert rows == N_CORES * ROWS_PER_CORE

    xf = np.ascontiguousarray(x.reshape(rows, IN_F).astype(np.float32))
    wt = np.ascontiguousarray(weight.T)  # [in_f, out_f] f32
    b2 = np.ascontiguousarray(bias.reshape(1, OUT_F))

    osl = OUT_F // N_CORES
    in_maps = []
    for c in range(N_CORES):
        xs = xf[c * ROWS_PER_CORE : (c + 1) * ROWS_PER_CORE]
        xtc = np.ascontiguousarray(xs.astype(np.float16).T)
        m = {
            "xt": xtc,
            "wt": wt,
            "bias": b2,
            "ws": np.ascontiguousarray(weight[c * osl : (c + 1) * osl, :]),
        }
        in_maps.append(m)

    nc = _get_nc()
    try:
        res = run_bass_kernel_spmd(nc, in_maps, core_ids=list(range(N_CORES)))
    except Exception:
        # transient device wedge (NRT_EXEC_UNIT_UNRECOVERABLE) — one retry
        import time

        time.sleep(5.0)
        res = run_bass_kernel_spmd(nc, in_maps, core_ids=list(range(N_CORES)))
    LAST_RESULTS = res
    y = np.concatenate(
        [res.results[c]["y"].astype(np.float32) for c in range(N_CORES)],
        axis=0,
    )
    return np.ascontiguousarray(y.reshape(b, s, OUT_F).astype(np.float32))
